# revision 46
# baseline (speedup 1.0000x reference)
"""Trainium2 Bass kernel for nn_MultiHeadAttention_66872640799208.

Math (per batch element b, S=2048, D=1024):
    qp = q @ Wq.T + bq ; kp = k @ Wk.T + bk ; vp = v @ Wv.T + bv
    scores = qp @ kp.T / D
    probs  = softmax(scores, axis=q)          # over the QUERY axis
    attn   = probs @ vp
    attn_w = softmax(attn, axis=q)            # over the sequence axis
    out    = (attn + q, attn_w)

Sharding: data-parallel over batch B=8 -> one batch element per NeuronCore,
no collectives.

Scores reassociation: qp.kp^T = q (Wq^T Wk) k^T + q.(Wq^T bk) + (bq^T Wk).k
+ bq.bk. The last two terms are constant along the softmax (query) axis and
cancel; the host precomputes M2 = Wq^T Wk and wqbk = Wq^T bk, so the kp
projection disappears entirely (raw kT is the scores lhsT, already in [d, s]
layout) and qp becomes u = q @ M2 (no bias). The alpha_i = q.wqbk term is
added into each scores psum group via a K=1 ones-broadcast matmul.

All big matmuls run in fp8e4m3 with MatmulPerfMode.DoubleRow: each
instruction contracts K=256 (two 128-partition tiles packed along the free
dim of both operands) -- 2x bf16 MACs/cycle on TRN2 hardware. Layouts (host
pre-transposes, casts to fp8):
  - qT/kT/vT [D, S] fp8: qT feeds the u projection + alpha matvec, kT is
    the scores stationary operand, vT feeds the vp projection,
  - u is produced in [b, q] fp8 layout so scoresT = kT.T @ u has the
    softmax axis (q) on the free dimension,
  - vp is produced in natural [s, e] bf16, then quantized to fp8 with the
    softmax-1 normalization folded in as (2048/Z_k) ~= 1.0 (fp8-safe; the
    leftover global 1/2048 moves into the exp scale of softmax #2 and a
    host-side divide of the residual output -- both exact).
The attn matmul psum therefore carries 2048*attn; the residual add uses
host-prescaled 2048*q (bf16) and the host divides the output by 2048.
The second softmax (over q, the partition axis) uses a ones-vector PE
matmul for column sums (bf16 expb -- fp8 there would put its quantization
noise directly on attn_w) and a K=1 PE matmul to broadcast 1/Z.

DMA: everything bulk rides the sync HWDGE queue, with the weight/aux
dispatches interleaved between the qT chunk dispatches so no input chunk
waits behind a 1MB weight transfer. qres rides the gpsimd SWDGE queue
(not latency-critical), outputs ride sync + scalar HWDGE queues.
"""

import sys

if "/opt/trn_rl_repo" not in sys.path:
    sys.path.insert(0, "/opt/trn_rl_repo")

import numpy as np
import ml_dtypes

B = 8
S = 2048
D = 1024
P = 128


def build_nc(s=S, d=D):
    """Build the single-core Bass program (SPMD: identical on all cores)."""
    import concourse.bass as bass
    import concourse.tile as tile
    from concourse import bacc, mybir

    bf16 = mybir.dt.bfloat16
    fp8 = mybir.dt.float8e4
    f32 = mybir.dt.float32
    DR = mybir.MatmulPerfMode.DoubleRow

    DT = d // P          # contraction tiles for projections / scores
    DH = DT // 2         # DoubleRow pairs over d
    ET = d // P          # e (feature) tiles
    ETA = ET + 1         # u tiles incl. the alpha column tile
    ST = s // P          # sequence tiles
    SH = ST // 2         # DoubleRow pairs over s (attn contraction)
    NFQ = min(512, s)    # matmul moving free-dim over q
    QC = s // NFQ        # q chunks
    NFD = min(512, d)    # matmul moving free-dim over d/e
    EC = d // NFD        # e chunks
    SCW = min(512, s)    # vT stream chunk width (in s)
    SC = s // SCW

    nc = bacc.Bacc("TRN2")

    # All tensors are host-permuted to partition-major contiguous layouts:
    # every DMA is 128 runs of >=2KB (minimal descriptor builds on the
    # dispatching engine, full DMA bandwidth). Outputs are un-permuted on
    # the host.
    qT = nc.dram_tensor("qT", [P, QC, DT, NFQ], fp8, kind="ExternalInput")
    kT = nc.dram_tensor("kT", [P, DT, s], fp8, kind="ExternalInput")
    vT = nc.dram_tensor("vT", [P, SC, DT, SCW], fp8, kind="ExternalInput")
    # m2 = [Wq^T Wk | Wq^T bk | 0-pad]: the alpha column rides the u
    # projection as tile ET, partition-row 0. Column-tile-major so the head
    # streams it in ETA small chunks ahead of the PE.
    m2 = nc.dram_tensor("m2", [P, ETA, DT, P], fp8, kind="ExternalInput")
    wvT = nc.dram_tensor("wvT", [P, DT, d], fp8, kind="ExternalInput")
    bv = nc.dram_tensor("bv", [d], f32, kind="ExternalInput")
    qres = nc.dram_tensor("qres", [P, ST, d], bf16, kind="ExternalInput")
    attn_o = nc.dram_tensor("attn", [P, ST, d], bf16, kind="ExternalOutput")
    attnw_o = nc.dram_tensor("attn_w", [P, ST, d], bf16,
                             kind="ExternalOutput")

    qT_r = qT[:]
    kT_r = kT[:]
    vT_r = vT[:]
    m2_r = m2[:]
    wv_r = wvT[:]
    qres_r = qres[:]
    attn_r = attn_o[:]
    attnw_r = attnw_o[:]

    with tile.TileContext(nc) as tc:
        with (
            tc.tile_pool(name="consts", bufs=1) as consts,
            tc.tile_pool(name="big", bufs=1) as big,
            tc.tile_pool(name="io", bufs=2) as io,
            tc.tile_pool(name="small", bufs=4) as small,
            tc.tile_pool(name="psum", bufs=4, space="PSUM") as psum,
            tc.tile_pool(name="psum1", bufs=1, space="PSUM") as psum1,
        ):
            m2_t = big.tile([P, ETA, DT, P], fp8, tag="W1")
            wv_t = big.tile([P, DT, d], fp8, tag="W2")
            bv_bc = consts.tile([P, d], f32)
            ones_col = consts.tile([P, 1], bf16)   # lhsT for column sums
            nc.vector.memset(ones_col[:], 1.0)
            # lhsT for the alpha broadcast as a K=1 DoubleRow pair: slot 0
            # zeroes out the junk partner row, slot 1 picks the alpha row
            ones_a2 = consts.tile([1, 2, P], fp8)
            nc.vector.memset(ones_a2[:, 0], 0.0)
            nc.vector.memset(ones_a2[:, 1], 1.0)
            ones_row = consts.tile([1, P], f32)    # lhsT for 1/Z broadcast
            nc.vector.memset(ones_row[:], 1.0)
            rz_all = consts.tile([P, ST], f32)     # per-k-row 1/Z of softmax 1
            rz2 = consts.tile([1, d], f32)         # 1/colsum of softmax 2
            rzb_sb = consts.tile([P, d], bf16)     # bcast 1/colsum, bf16

            u = big.tile([P, ETA, s], fp8, tag="A")         # uT: [b, q] + alpha
            kT_sb = big.tile([P, DT, s], fp8, tag="B")      # raw kT
            qT_sb = big.tile([P, QC, DT, NFQ], fp8, tag="Q")  # raw qT
            vp16 = big.tile([P, ST, d], bf16, tag="V2")     # natural [s, e]
            vp8 = big.tile([P, ST, d], fp8, tag="V")        # *(2048/Z_k)

            # Head DMA plan. The PE's first matmul needs only m2's first
            # column-tile + qT's first chunk (~0.6MB): both are chunked and
            # ride separate queues, with everything later (kT, wv, bv)
            # queued strictly behind so it can't steal head bandwidth.
            #   sync:   m2 et-chunks -> kT -> wv -> (vT chunks, 1b loop)
            #   scalar: qT qc-chunks
            #   gpsimd: bv (+ qres in phase 3)
            for et in range(ETA):
                nc.sync.dma_start(out=m2_t[:, et], in_=m2_r[:, et])
            for qc in range(QC):
                nc.scalar.dma_start(out=qT_sb[:, qc], in_=qT_r[:, qc])
            nc.sync.dma_start(out=kT_sb[:], in_=kT_r)
            nc.sync.dma_start(out=wv_t[:], in_=wv_r)
            bv_ap = bv[:]
            nc.gpsimd.dma_start(
                out=bv_bc[:],
                in_=bass.AP(
                    tensor=bv_ap.tensor,
                    offset=bv_ap.offset,
                    ap=[[0, P], [1, d]],
                ),
            )

            # ---- Phase 1a: u projection (alpha = col ET, partition-row 0) --
            for qc in range(QC):
                for et in range(ETA):
                    ps = psum.tile([P, NFQ], f32, tag="ps")
                    for j in range(DH):
                        nc.tensor.matmul(
                            ps[:],
                            m2_t[:, et, 2 * j:2 * j + 2, :],
                            qT_sb[:, qc, 2 * j:2 * j + 2, :],
                            start=(j == 0),
                            stop=(j == DH - 1),
                            perf_mode=DR,
                        )
                    nc.scalar.copy(
                        out=u[:, et, qc * NFQ:(qc + 1) * NFQ], in_=ps[:]
                    )

            # ---- Phase 1b: vp projection (natural layout, bf16) ----
            for sc in range(SC):
                vt = io.tile([P, DT, SCW], fp8, tag="xin")
                nc.sync.dma_start(out=vt[:], in_=vT_r[:, sc])
                for sti in range(SCW // P):
                    st = sc * (SCW // P) + sti
                    for ec in range(EC):
                        ps = psum.tile([P, NFD], f32, tag="ps")
                        for j in range(DH):
                            nc.tensor.matmul(
                                ps[:],
                                vt[:, 2 * j:2 * j + 2, sti * P:(sti + 1) * P],
                                wv_t[:, 2 * j:2 * j + 2,
                                     ec * NFD:(ec + 1) * NFD],
                                start=(j == 0),
                                stop=(j == DH - 1),
                                perf_mode=DR,
                            )
                        nc.vector.tensor_add(
                            out=vp16[:, st, ec * NFD:(ec + 1) * NFD],
                            in0=ps[:],
                            in1=bv_bc[:, ec * NFD:(ec + 1) * NFD],
                        )

            # ---- Phase 2: scoresT -> softmax over q -> probs (fp8) ----
            # scoresT = kT.T @ u + 1^T alpha; probs reuses the weights' slot.
            # No max-subtraction: |scores/d| < ~0.3 by construction.
            probs = big.tile([P, ST, s], fp8, tag="PR")     # [k, q] per k-tile
            for kt in range(ST):
                for qc in range(QC):
                    ps = psum.tile([P, NFQ], f32, tag="ps")
                    for j in range(DH):
                        nc.tensor.matmul(
                            ps[:],
                            kT_sb[:, 2 * j:2 * j + 2, kt * P:(kt + 1) * P],
                            u[:, 2 * j:2 * j + 2, qc * NFQ:(qc + 1) * NFQ],
                            start=(j == 0),
                            stop=False,
                            perf_mode=DR,
                        )
                    nc.tensor.matmul(
                        ps[:],
                        ones_a2[:],
                        u[0:1, ET - 1:ET + 1, qc * NFQ:(qc + 1) * NFQ],
                        start=False,
                        stop=True,
                        perf_mode=DR,
                    )
                    nc.scalar.activation(
                        out=probs[:, kt, qc * NFQ:(qc + 1) * NFQ],
                        in_=ps[:],
                        func=mybir.ActivationFunctionType.Exp,
                        scale=1.0 / d,
                    )
                # Z from the quantized probs on the DVE — keeps the scalar
                # engine off the ACTIVATION_READ_ACCUMULATOR path (the fp8
                # reduce is the slow 1x path but fits under the PE phase)
                zsum = small.tile([P, 1], f32, tag="zsum")
                nc.vector.reduce_sum(
                    out=zsum[:], in_=probs[:, kt, :], axis=mybir.AxisListType.X
                )
                nc.vector.reciprocal(out=rz_all[:, kt:kt + 1], in_=zsum[:])
                # quantize vp to fp8 with (2048/Z_k) folded into its k-rows:
                # ~1.0 so the fp8 range is preserved; the global 1/2048 is
                # repaid at the attn psum (exp scale + host divide).
                nc.vector.tensor_scalar(
                    out=vp8[:, kt, :],
                    in0=vp16[:, kt, :],
                    scalar1=rz_all[:, kt:kt + 1],
                    scalar2=float(s),
                    op0=mybir.AluOpType.mult,
                    op1=mybir.AluOpType.mult,
                )

            # ---- Phase 3: 2048*attn = probsT.T @ vp8 ; residual; exp ----
            # expb reuses u's slot (tag A).
            expb = big.tile([P, ST, d], bf16, tag="A")      # exp(attn), bf16
            cs_ps = psum1.tile([1, d], f32, tag="cs")       # colsums of exp
            for st in range(ST):
                qres_t = io.tile([P, d], bf16, tag="xin")
                # gpsimd SWDGE queue: keeps the sync queue free for attn out
                nc.gpsimd.dma_start(out=qres_t[:], in_=qres_r[:, st, :])
                ao = io.tile([P, d], bf16, tag="ao")
                for ec in range(EC):
                    ps = psum.tile([P, NFD], f32, tag="ps")
                    for j in range(SH):
                        nc.tensor.matmul(
                            ps[:],
                            probs[:, 2 * j:2 * j + 2, st * P:(st + 1) * P],
                            vp8[:, 2 * j:2 * j + 2, ec * NFD:(ec + 1) * NFD],
                            start=(j == 0),
                            stop=(j == SH - 1),
                            perf_mode=DR,
                        )
                    nc.vector.tensor_add(
                        out=ao[:, ec * NFD:(ec + 1) * NFD],
                        in0=ps[:],
                        in1=qres_t[:, ec * NFD:(ec + 1) * NFD],
                    )
                    nc.scalar.activation(
                        out=expb[:, st, ec * NFD:(ec + 1) * NFD],
                        in_=ps[:],
                        func=mybir.ActivationFunctionType.Exp,
                        scale=1.0 / s,
                    )
                    nc.tensor.matmul(
                        cs_ps[:, ec * NFD:(ec + 1) * NFD],
                        ones_col[:],
                        expb[:, st, ec * NFD:(ec + 1) * NFD],
                        start=(st == 0),
                        stop=(st == ST - 1),
                    )
                nc.sync.dma_start(out=attn_r[:, st, :], in_=ao[:])

            # ---- Phase 3.5: 1/colsum, broadcast across partitions ----
            # approx recip: ~51 ULP, ~5x faster; Z ~ s +- 5% is edge-case-safe
            nc.vector.reciprocal_approx_fast(out=rz2[:], in_=cs_ps[:])
            rzb = psum1.tile([P, d], f32, tag="cs")         # reuses cs_ps bank
            for ec in range(EC):
                nc.tensor.matmul(
                    rzb[:, ec * NFD:(ec + 1) * NFD],
                    ones_row[:],
                    rz2[:, ec * NFD:(ec + 1) * NFD],
                    start=True,
                    stop=True,
                )
            # bf16 copy so the phase-4 multiplies run at 2x 16-bit DVE rate
            nc.scalar.copy(out=rzb_sb[:], in_=rzb[:])

            # ---- Phase 4: attn_w = exp(attn) * (1/colsum) ----
            # Pure tail (depends on the global colsum). Per-s-tile multiplies
            # with plain unit-stride all-bf16 APs (the DVE 2x 16-bit fast
            # path; a stride-0 broadcast AP fell back to the 1x path), DMAs
            # in 2-tile groups alternating the sync/scalar HWDGE queues,
            # 4 groups of buffering so the mul chain never waits on a DMA
            # completion semaphore.
            NB = min(2, ST)
            NSLOT = min(4 * NB, ST)
            aw_all = big.tile([P, NSLOT, d], bf16, tag="B")
            for st in range(ST):
                nc.vector.tensor_mul(
                    out=aw_all[:, st % NSLOT, :],
                    in0=expb[:, st, :],
                    in1=rzb_sb[:],
                )
                if st % NB == NB - 1:
                    g = st // NB
                    lo = (g * NB) % NSLOT
                    dma_eng = nc.sync if g % 2 == 0 else nc.scalar
                    dma_eng.dma_start(
                        out=attnw_r[:, g * NB:(g + 1) * NB, :],
                        in_=aw_all[:, lo:lo + NB, :],
                    )

    return nc


def _build_m2(Wq, Wk, bk):
    """[Wq^T Wk | Wq^T bk | 0-pad to a full 128-col tile] in float64."""
    d = Wq.shape[0]
    m2 = np.zeros((d, d + P), dtype=np.float64)
    m2[:, :d] = Wq.T @ Wk
    m2[:, d] = Wq.T @ bk
    return m2


def _pmajor_ct(xT):
    """[C, n] (contraction-major) -> [P, C//P, n]: partition-major so the
    SBUF-bound DMA is 128 contiguous runs."""
    C, n = xT.shape
    return np.ascontiguousarray(xT.reshape(C // P, P, n).transpose(1, 0, 2))


def _pmajor_ct_chunked(xT, nchunk):
    """[C, n] -> [P, n//nchunk, C//P, nchunk]: chunk-major along n so each
    chunk is one contiguous-per-partition DMA."""
    C, n = xT.shape
    return np.ascontiguousarray(
        xT.reshape(C // P, P, n // nchunk, nchunk).transpose(1, 2, 0, 3)
    )


def _pmajor_rows(x):
    """[R, n] (row-tiled) -> [P, R//P, n] with row = st*P + p."""
    R, n = x.shape
    return np.ascontiguousarray(x.reshape(R // P, P, n).transpose(1, 0, 2))


def _unpmajor_rows(xp):
    """[P, ST, n] -> [ST*P, n] with row = st*P + p."""
    Pp, STt, n = xp.shape
    return np.ascontiguousarray(xp.transpose(1, 0, 2)).reshape(STt * Pp, n)


def _host_prep(q, k, v, Wq, bq, Wk, bk, Wv, bv):
    """Shard over batch, fold Wq/Wk/bk into M2, permute to partition-major,
    cast."""
    fp8 = ml_dtypes.float8_e4m3
    bf16 = ml_dtypes.bfloat16
    s = q.shape[1]
    SCW = min(512, s)
    NFQ = min(512, s)
    q = np.asarray(q, dtype=np.float32)
    k = np.asarray(k, dtype=np.float32)
    v = np.asarray(v, dtype=np.float32)
    Wq = np.asarray(Wq, dtype=np.float64)
    Wk = np.asarray(Wk, dtype=np.float64)
    bk = np.asarray(bk, dtype=np.float64)
    m2 = _pmajor_ct_chunked(_build_m2(Wq, Wk, bk).astype(fp8), P)
    wvT = _pmajor_ct(np.asarray(Wv, dtype=np.float32).T.astype(fp8))
    bv = np.ascontiguousarray(np.asarray(bv, dtype=np.float32))

    in_maps = []
    for i in range(B):
        in_maps.append(
            {
                "qT": _pmajor_ct_chunked(q[i].T.astype(fp8), NFQ),
                "kT": _pmajor_ct(k[i].T.astype(fp8)),
                "vT": _pmajor_ct_chunked(v[i].T.astype(fp8), SCW),
                "m2": m2,
                "wvT": wvT,
                "bv": bv,
                "qres": _pmajor_rows((q[i] * float(s)).astype(bf16)),
            }
        )
    return in_maps


def _host_post(attn_raw, attnw_raw):
    """Un-permute each core's [P, ST, d] output, undo the s*psum scaling,
    widen to f32. The scale equals the sequence length ST*P."""
    s_scale = float(attn_raw.shape[-2] * attn_raw.shape[-3])
    attn = np.stack([_unpmajor_rows(a) for a in attn_raw])
    attn_w = np.stack([_unpmajor_rows(a) for a in attnw_raw])
    attn = attn.astype(np.float32) * (1.0 / s_scale)
    attn_w = attn_w.astype(np.float32)
    return attn, attn_w


_CACHED_NC = None


def kernel(q, k, v, Wq, bq, Wk, bk, Wv, bv):
    global _CACHED_NC
    from concourse import bass_utils

    in_maps = _host_prep(q, k, v, Wq, bq, Wk, bk, Wv, bv)
    if _CACHED_NC is None:
        _CACHED_NC = build_nc()
        _CACHED_NC.finalize()  # bacc passes (reg alloc, wait splitting)
    res = bass_utils.run_bass_kernel_spmd(
        _CACHED_NC, in_maps, core_ids=list(range(B))
    )
    attn = np.stack([np.asarray(res.results[i]["attn"]) for i in range(B)])
    attn_w = np.stack([np.asarray(res.results[i]["attn_w"]) for i in range(B)])
    return _host_post(attn, attn_w)
